# revision 27
# baseline (speedup 1.0000x reference)
"""Trainium2 Bass kernel for nn_CustomDecoderLayer (FAVOR+ decoder layer).

Sharding: 8 cores = 4 batch groups x 2-way tensor parallel (heads/ffn),
pair all-reduces after Wo and W2. Per-core program is SPMD-identical.
Activations are F-layout on chip (features on partitions, seq on free dim).

v2: all weights bf16 + SBUF-resident (one DMA each), weight-stationary
matmul loops (Ldweights dedup), bf16 residual trunk kept in SBUF, 2-way
chunked all-reduces with CA memory-side (k/v + phi_k + kv summary)
overlapped with AR1.
"""
import os
import sys
sys.path.insert(0, "/opt/trn_rl_repo")
from contextlib import ExitStack

import numpy as np
import ml_dtypes

import concourse.bass as bass
import concourse.mybir as mybir
import concourse.tile as tile
from concourse import bacc, bass_isa

f32 = mybir.dt.float32
f32r = mybir.dt.float32r
bf16 = mybir.dt.bfloat16
AF = mybir.ActivationFunctionType
AX = mybir.AxisListType
ALU = mybir.AluOpType

D, H, DH, M = 1024, 16, 64, 256
S, B, F = 2048, 4, 4096
HL, FL = 8, 2048
C2 = 0.5 * (DH ** -0.5)      # 0.0625, exact in bf16
EPS16 = 1.0e-6 * 16.0
KD = D // 128                # 8
NCH = 4                      # col chunks of 512
RT = S // 128                # 16
SH = S // 2                  # 1024, AR chunk width
RG = [[0, 1], [2, 3], [4, 5], [6, 7]]

_CACHE = {}
KREPS = int(os.environ.get("KREPS", "1"))
TLSIM = os.environ.get("TLSIM", "0") == "1"


def _allreduce(nc, cc_in, cc_out):
    """Pair AllReduce; under TLSIM replaced by a DRAM copy (timing sim only)."""
    if TLSIM:
        nc.sync.dma_start(out=cc_out[:], in_=cc_in[:])
    else:
        nc.gpsimd.collective_compute("AllReduce", ALU.add, replica_groups=RG,
                                     ins=[cc_in.opt()], outs=[cc_out.opt()])


def _ln(nc, tc, ctx, x_t, g_t, b_t, out_t, c_invd, c_eps, chunks=range(NCH),
        out_off=0):
    """LayerNorm F-layout: x_t (128, KD, S) bf16 -> out_t bf16.

    Reads x_t chunks (global 512-col indices); writes out_t at column
    ch*512 - out_off (so a half-sized out tile starts at 0)."""
    ps = ctx.enter_context(tc.tile_pool(name="lnps", bufs=2, space="PSUM"))
    sb = ctx.enter_context(tc.tile_pool(name="lnsb", bufs=2))
    for ch in chunks:
        cs = bass.ts(ch, 512)
        os_ = bass.ds(ch * 512 - out_off, 512)
        mv = ps.tile([128, 2, 512], f32, tag="ln_ps", name="ln_ps")
        for kd in range(KD):
            nc.tensor.matmul(mv[:, 0, :], c_invd[:], x_t[:, kd, cs],
                             start=(kd == 0), stop=(kd == KD - 1),
                             skip_group_check=True)
        for kd in range(KD):
            x2 = sb.tile([128, 512], bf16, tag="ln_x2", name="ln_x2")
            nc.gpsimd.tensor_mul(x2[:], x_t[:, kd, cs], x_t[:, kd, cs])
            nc.tensor.matmul(mv[:, 1, :], c_invd[:], x2[:],
                             start=(kd == 0), stop=(kd == KD - 1),
                             skip_group_check=True)
        mu = sb.tile([128, 512], bf16, tag="ln_mu", name="ln_mu")
        nc.scalar.copy(mu[:], mv[:, 0, :])
        mu2 = sb.tile([128, 512], f32, tag="ln_mu2", name="ln_mu2")
        nc.vector.tensor_mul(mu2[:], mu[:], mu[:])
        var = sb.tile([128, 512], f32, tag="ln_var", name="ln_var")
        nc.vector.tensor_sub(var[:], mv[:, 1, :], mu2[:])
        lv = sb.tile([128, 512], f32, tag="ln_lv", name="ln_lv")
        nc.scalar.activation(lv[:], var[:], AF.Ln, bias=c_eps[:])
        rstd = sb.tile([128, 512], bf16, tag="ln_rstd", name="ln_rstd")
        nc.scalar.activation(rstd[:], lv[:], AF.Exp, scale=-0.5)
        for kd in range(KD):
            xm = sb.tile([128, 512], bf16, tag="ln_xm", name="ln_xm")
            nc.vector.tensor_sub(xm[:], x_t[:, kd, cs], mu[:])
            nc.vector.tensor_mul(out_t[:, kd, os_], xm[:], rstd[:])


def _ln2h(nc, tc, ctx, x_t, g_t, b_t, out_h, c_invd, c_eps, half):
    """LN of S-half `half` of x_t into half-width out tile (128, KD, SH)."""
    _ln(nc, tc, ctx, x_t, g_t, b_t, out_h, c_invd, c_eps,
        chunks=range(2 * half, 2 * half + 2), out_off=half * SH)


def _proj_qk(nc, tc, ctx, src_t, w_sb, b_t, out_t, ps, s_off=0, s_len=S):
    """out_t[:, mt, s_off:s_off+s_len] bf16 = (src.T @ W).T over s range.

    src_t is (128, KD, s_len); w_sb (128, KD, 512) bf16 resident."""
    nch = s_len // 512
    for mt in range(4):
        o_ps = ps.tile([128, nch, 512], f32, tag="proj_ps", name="proj_ps")
        for kd in range(KD):
            for ch in range(nch):
                nc.tensor.matmul(o_ps[:, ch, :],
                                 w_sb[:, kd, mt * 128:(mt + 1) * 128],
                                 src_t[:, kd, bass.ts(ch, 512)],
                                 start=(kd == 0), stop=(kd == KD - 1),
                                 skip_group_check=True)
        for ch in range(nch):
            nc.scalar.activation(
                out_t[:, mt, bass.ds(s_off + ch * 512, 512)],
                o_ps[:, ch, :], AF.Identity, bias=b_t[:, mt:mt + 1])


def _proj_v(nc, tc, ctx, src_t, wv_sb, bv_rep, v_aug, ps):
    """v R-layout -> v_aug (128, RT, HL, 65) bf16 (last col preset to 1)."""
    for rt in range(RT):
        v_ps = ps.tile([128, 512], f32, tag="v_ps", name="v_ps")
        for kd in range(KD):
            nc.tensor.matmul(v_ps[:], src_t[:, kd, rt * 128:(rt + 1) * 128],
                             wv_sb[:, kd, :], start=(kd == 0),
                             stop=(kd == KD - 1), skip_group_check=True)
        nc.vector.tensor_add(v_aug[:, rt, :, 0:64], v_ps[:], bv_rep[:])


def _phi_k_side(nc, tc, ctx, kT, v_aug, wf_pair, c_negblk, ident_bf, kvT_t,
                s_t):
    """FAVOR+ k side: E = exp(proj - sq) unstabilized in f32 PSUM; the global
    stab scale e^{-max} and the eps*sum(v) term are applied at the tiny kva
    stage, matching the reference eps weighting exactly."""
    ps_pj = ctx.enter_context(tc.tile_pool(name="kps_pj", bufs=3, space="PSUM"))
    ps_ns = ctx.enter_context(tc.tile_pool(name="kps_ns", bufs=2, space="PSUM"))
    ps_kv = ctx.enter_context(tc.tile_pool(name="kps_kv", bufs=2, space="PSUM"))
    ps_tp = ctx.enter_context(tc.tile_pool(name="kps_tp", bufs=1, space="PSUM"))
    sb = ctx.enter_context(tc.tile_pool(name="kphisb", bufs=6))
    big = ctx.enter_context(tc.tile_pool(name="kphibig", bufs=1))

    for p in range(4):
        E_t = big.tile([128, RT, 2, 257], bf16, tag="E_t", name="E_t")
        nc.vector.memset(E_t[:, :, :, 256:257], EPS16)
        stab_run = sb.tile([128, 2], f32, tag="stab_run", name="stab_run")
        for rt in range(RT):
            rs = bass.ts(rt, 128)
            k2 = sb.tile([128, 128], bf16, tag="k2", name="k2")
            nc.gpsimd.tensor_mul(k2[:], kT[:, p, rs], kT[:, p, rs])
            pj = ps_pj.tile([128, 2, 256], f32, tag="pj", name="pj_k")
            nc.tensor.matmul(pj[:, :, :], kT[:, p, rs], wf_pair[:])
            nsq = ps_ns.tile([128, 2], f32, tag="nsq", name="nsq_k")
            nc.tensor.matmul(nsq[:], k2[:], c_negblk[:])
            rmax = sb.tile([128, 2], f32, tag="rmax", name="rmax_k")
            nc.vector.reduce_max(rmax[:], pj[:], axis=AX.X)
            if rt == 0:
                nc.vector.tensor_copy(stab_run[:], rmax[:])
            else:
                nc.vector.tensor_max(stab_run[:], stab_run[:], rmax[:])
            nsq_sb = sb.tile([128, 2], f32, tag="nsq_sb", name="nsq_sb")
            nc.scalar.copy(nsq_sb[:], nsq[:])
            for h in range(2):
                nc.scalar.activation(E_t[:, rt, h, 0:256], pj[:, h, :],
                                     AF.Exp, bias=nsq_sb[:, h:h + 1])
        stab_rep = sb.tile([128, 2], f32, tag="stab_rep", name="stab_rep")
        nc.gpsimd.partition_all_reduce(stab_rep[:], stab_run[:], channels=128,
                                       reduce_op=bass_isa.ReduceOp.max)
        nc.scalar.activation(s_t[:, p, :], stab_rep[:], AF.Exp, scale=-1.0)
        for h in range(2):
            kv_ps = ps_kv.tile([65, 257], f32, tag="kv_ps", name="kv_ps")
            for rt in range(RT):
                nc.tensor.matmul(kv_ps[:], v_aug[:, rt, p * 2 + h, :],
                                 E_t[:, rt, h, :], start=(rt == 0),
                                 stop=(rt == RT - 1), skip_group_check=True)
            csum = sb.tile([65, 1], f32, tag="csum", name="csum")
            nc.vector.tensor_copy(csum[:], kv_ps[:, 256:257])
            kva = sb.tile([65, 256], bf16, tag="kva", name="kva")
            nc.vector.tensor_scalar(kva[:], kv_ps[:, 0:256],
                                    s_t[0:65, p, h:h + 1], csum[:],
                                    ALU.mult, ALU.add)
            for mt in range(2):
                tp = ps_tp.tile([128, 65], bf16, tag="tp_kv", name="tp_kv")
                nc.tensor.transpose(tp[:], kva[0:65, mt * 128:(mt + 1) * 128],
                                    ident_bf[0:65, 0:65])
                nc.scalar.copy(kvT_t[:, p, h, mt, :], tp[:])


def _phi_q_side(nc, tc, ctx, qT, wf_pair, c_negblk, ident_bf, kvT_t, attn_t,
                c_eps16):
    """FAVOR+ q side: phi(q), out/z -> attn_t (128, 4, S) bf16."""
    ps_pj = ctx.enter_context(tc.tile_pool(name="qps_pj", bufs=2, space="PSUM"))
    ps_ns = ctx.enter_context(tc.tile_pool(name="qps_ns", bufs=1, space="PSUM"))
    ps_tp = ctx.enter_context(tc.tile_pool(name="qps_tp", bufs=1, space="PSUM"))
    ps_o = ctx.enter_context(tc.tile_pool(name="qps_o", bufs=3, space="PSUM"))
    sb = ctx.enter_context(tc.tile_pool(name="qphisb", bufs=6))
    big = ctx.enter_context(tc.tile_pool(name="qphibig", bufs=2))

    for p in range(4):
        pqT = big.tile([128, 2, 2, S], bf16, tag="pqT", name="pqT")
        for rt in range(RT):
            rs = bass.ts(rt, 128)
            q2 = sb.tile([128, 128], bf16, tag="q2", name="q2")
            nc.gpsimd.tensor_mul(q2[:], qT[:, p, rs], qT[:, p, rs])
            pj = ps_pj.tile([128, 2, 256], f32, tag="pj", name="pj_q")
            nc.tensor.matmul(pj[:, :, :], qT[:, p, rs], wf_pair[:])
            nsq = ps_ns.tile([128, 2], f32, tag="nsq", name="nsq_q")
            nc.tensor.matmul(nsq[:], q2[:], c_negblk[:])
            rmax = sb.tile([128, 2], f32, tag="rmax", name="rmax_q")
            nc.vector.reduce_max(rmax[:], pj[:], axis=AX.X)
            bias_q = sb.tile([128, 2], f32, tag="bias_q", name="bias_q")
            nc.vector.tensor_sub(bias_q[:], nsq[:], rmax[:])
            pqR = sb.tile([128, 2, 256], bf16, tag="pqR", name="pqR")
            for h in range(2):
                nc.scalar.activation(pqR[:, h, :], pj[:, h, :], AF.Exp,
                                     bias=bias_q[:, h:h + 1])
            for h in range(2):
                for mt in range(2):
                    tp = ps_tp.tile([128, 128], bf16, tag="tp_pq",
                                    name="tp_pq")
                    nc.tensor.transpose(tp[:],
                                        pqR[:, h, mt * 128:(mt + 1) * 128],
                                        ident_bf[:])
                    if mt == 0:
                        nc.vector.tensor_scalar_add(pqT[:, h, mt, rs], tp[:],
                                                    EPS16)
                    else:
                        nc.scalar.activation(pqT[:, h, mt, rs], tp[:],
                                             AF.Identity, bias=c_eps16[:])
        for ch in range(NCH):
            cs = bass.ts(ch, 512)
            for h in range(2):
                hp = slice(64 * h, 64 * h + 64)
                o_ps = ps_o.tile([65, 512], f32, tag="o_ps", name="o_ps")
                for mt in range(2):
                    nc.tensor.matmul(o_ps[:, :], kvT_t[:, p, h, mt, 0:65],
                                     pqT[:, h, mt, cs], start=(mt == 0),
                                     stop=(mt == 1), skip_group_check=True)
                zr = sb.tile([1, 512], f32, tag="zr", name="zr")
                nc.vector.reciprocal(zr[:], o_ps[64:65, :])
                zb = sb.tile([128, 512], f32, tag="zb", name="zb")
                nc.gpsimd.partition_broadcast(zb[:], zr[:], channels=128)
                nc.vector.tensor_mul(attn_t[hp, p, cs], o_ps[0:64, :],
                                     zb[0:64, :])


def _wo_ar(nc, tc, ctx, attn_t, wo_sb, bo_t, cc_i, cc_o):
    """Wo projection, per S-half: write cc_in half, AllReduce it."""
    ps = ctx.enter_context(tc.tile_pool(name="wops", bufs=2, space="PSUM"))
    sb = ctx.enter_context(tc.tile_pool(name="wosb", bufs=2))
    for half in range(2):
        for md in range(KD):
            o_ps = ps.tile([128, 2, 512], f32, tag="wo_ps", name="wo_ps")
            for k4 in range(4):
                for c2 in range(2):
                    cs = bass.ds(half * SH + c2 * 512, 512)
                    nc.tensor.matmul(o_ps[:, c2, :],
                                     wo_sb[:, k4, md * 128:(md + 1) * 128],
                                     attn_t[:, k4, cs],
                                     start=(k4 == 0), stop=(k4 == 3),
                                     skip_group_check=True)
            ev = sb.tile([128, 1024], bf16, tag="wo_ev", name="wo_ev")
            for c2 in range(2):
                nc.scalar.activation(ev[:, c2 * 512:(c2 + 1) * 512],
                                     o_ps[:, c2, :], AF.Identity,
                                     bias=bo_t[:, md:md + 1])
            nc.sync.dma_start(
                out=cc_i[md * 128:(md + 1) * 128,
                         bass.ds(half * SH, SH)],
                in_=ev[:])
    _allreduce(nc, cc_i, cc_o)


def _join(nc, tc, ctx, x_t, cc_o, half):
    """x_t[:, :, half] += AR output half (bf16)."""
    sb = ctx.enter_context(tc.tile_pool(name="joinsb", bufs=3))
    hs = bass.ds(half * SH, SH)
    for kd in range(KD):
        ar = sb.tile([128, SH], bf16, tag="jar", name="jar")
        nc.sync.dma_start(out=ar[:], in_=cc_o[kd * 128:(kd + 1) * 128, hs])
        nc.vector.tensor_add(x_t[:, kd, hs], x_t[:, kd, hs], ar[:])


def build_nc():
    nc = bacc.Bacc("TRN2", target_bir_lowering=False, debug=False,
                   num_devices=8)

    def din(name, shape, dtype=f32):
        return nc.dram_tensor(name, list(shape), dtype,
                              kind="ExternalInput").ap()

    xT = din("xT", (D, S), bf16)
    memT = din("memT", (D, S), bf16)
    wg = {}
    for pre in ("sa", "ca"):
        wg[pre + "_wq"] = din(pre + "_wq", (128, KD, 512), bf16)
        wg[pre + "_wk"] = din(pre + "_wk", (128, KD, 512), bf16)
        wg[pre + "_wv"] = din(pre + "_wv", (128, KD, 512), bf16)
        wg[pre + "_wo"] = din(pre + "_wo", (128, 4, D), bf16)
        wg[pre + "_bq"] = din(pre + "_bq", (128, 4))
        wg[pre + "_bk"] = din(pre + "_bk", (128, 4))
        wg[pre + "_bv"] = din(pre + "_bv", (1, 512))
        wg[pre + "_bo"] = din(pre + "_bo", (128, 8))
        wg[pre + "_wf"] = din(pre + "_wf", (64, 256), bf16)
    w1_d = din("w1", (128, KD, FL), bf16)
    w2_d = din("w2", (128, 16, D), bf16)
    b1_d = din("b1", (128, 16))
    b2_d = din("b2", (128, 8))
    ln_d = {}
    for i in ("1", "2", "3"):
        ln_d["g" + i] = din("ln%s_g" % i, (128, 8))
        ln_d["b" + i] = din("ln%s_b" % i, (128, 8))
    c_invd_d = din("c_invd", (128, 128), bf16)
    c_negblk_d = din("c_negblk", (128, 2), bf16)
    ident_d = din("c_ident", (128, 128), bf16)

    outT = nc.dram_tensor("outT", [D, S], f32, kind="ExternalOutput").ap()

    with tile.TileContext(nc) as tc:
        with ExitStack() as top:
            dram = top.enter_context(tc.tile_pool(name="dram", bufs=1,
                                                  space="DRAM"))
            ccs = {}
            for i in ("1", "2", "3"):
                ccs["in" + i] = dram.tile([D, S], bf16, name="cc_in" + i)
                ccs["out" + i] = dram.tile([D, S], bf16, name="cc_out" + i)

            const = top.enter_context(tc.tile_pool(name="const", bufs=1))
            c_invd = const.tile([128, 128], bf16, name="c_invd")
            nc.sync.dma_start(out=c_invd[:], in_=c_invd_d[:])
            c_negblk = const.tile([128, 2], bf16, name="c_negblk")
            nc.sync.dma_start(out=c_negblk[:], in_=c_negblk_d[:])
            ident_bf = const.tile([128, 128], bf16, name="ident_bf")
            nc.sync.dma_start(out=ident_bf[:], in_=ident_d[:])
            c_eps = const.tile([128, 1], f32, name="c_eps")
            nc.vector.memset(c_eps[:], 1.0e-5)
            c_eps16 = const.tile([128, 1], f32, name="c_eps16")
            nc.vector.memset(c_eps16[:], EPS16)
            cb = {}
            for pre in ("sa", "ca"):
                for nm in ("bq", "bk", "bo"):
                    t = const.tile(list(wg[pre + "_" + nm].shape), f32,
                                   name=pre + nm)
                    nc.sync.dma_start(out=t[:], in_=wg[pre + "_" + nm][:])
                    cb[pre + "_" + nm] = t
                t = const.tile([1, 512], f32, name=pre + "bv")
                nc.sync.dma_start(out=t[:], in_=wg[pre + "_bv"][:])
                cb[pre + "_bv"] = t
                wff = const.tile([128, 512], bf16, name=pre + "wff")
                nc.vector.memset(wff[64:128, 0:256], 0.0)
                nc.vector.memset(wff[0:64, 256:512], 0.0)
                nc.sync.dma_start(out=wff[0:64, 0:256], in_=wg[pre + "_wf"][:])
                nc.sync.dma_start(out=wff[64:128, 256:512],
                                  in_=wg[pre + "_wf"][:])
                cb[pre + "_wf"] = wff
            for nm, d_ap in (("b1", b1_d), ("b2", b2_d)):
                t = const.tile(list(d_ap.shape), f32, name=nm)
                nc.sync.dma_start(out=t[:], in_=d_ap[:])
                cb[nm] = t
            for k, d_ap in ln_d.items():
                t = const.tile([128, 8], f32, name="ln" + k)
                nc.sync.dma_start(out=t[:], in_=d_ap[:])
                cb["ln" + k] = t

            def load_w(pool, d_ap, name):
                t = pool.tile(list(d_ap.shape), bf16, name=name)
                nc.sync.dma_start(out=t[:], in_=d_ap[:])
                return t

            for _rep in range(KREPS):
                # ---------- residual trunk, resident bf16
                xs_stack = ExitStack()
                x_t = xs_stack.enter_context(
                    tc.tile_pool(name="xpool", bufs=1, side="right")).tile(
                        [128, KD, S], bf16, name="x_t")
                for kd in range(KD):
                    nc.sync.dma_start(out=x_t[:, kd, :],
                                      in_=xT[kd * 128:(kd + 1) * 128, :])

                # ---------- LN1 -> t2 ; SA block
                sa_s = ExitStack()
                wsa_p = sa_s.enter_context(tc.tile_pool(name="wsa", bufs=1))
                sa_wq = load_w(wsa_p, wg["sa_wq"], "sa_wq")
                sa_wk = load_w(wsa_p, wg["sa_wk"], "sa_wk")
                sa_wv = load_w(wsa_p, wg["sa_wv"], "sa_wv")
                sa_wo = load_w(wsa_p, wg["sa_wo"], "sa_wo")
                sa_res = sa_s.enter_context(tc.tile_pool(name="sares", bufs=1))
                qT = sa_res.tile([128, 4, S], bf16, name="sa_qT")
                kT = sa_res.tile([128, 4, S], bf16, name="sa_kT")
                v_aug = sa_res.tile([128, RT, HL, 65], bf16, name="sa_vaug")
                nc.vector.memset(v_aug[:, :, :, 64:65], 1.0)
                attn_t = sa_res.tile([128, 4, S], bf16, name="sa_attn")
                kvT_t = sa_res.tile([128, 4, 2, 2, 65], bf16, name="sa_kvT")
                s_t = sa_res.tile([128, 4, 2], f32, name="sa_s_t")
                bv_rep = sa_res.tile([128, 512], f32, name="sa_bvrep")
                nc.gpsimd.partition_broadcast(bv_rep[:], cb["sa_bv"][:],
                                              channels=128)
                with ExitStack() as ph:
                    t2p = ph.enter_context(tc.tile_pool(name="t2p1", bufs=1,
                                                        side="right"))
                    t2_t = t2p.tile([128, KD, S], bf16, name="t2_t1")
                    with ExitStack() as lns:
                        _ln(nc, tc, lns, x_t, cb["lng1"], cb["lnb1"], t2_t,
                            c_invd, c_eps)
                    with ExitStack() as prj:
                        ps = prj.enter_context(
                            tc.tile_pool(name="prjps", bufs=1, space="PSUM"))
                        psv = prj.enter_context(
                            tc.tile_pool(name="prjpsv", bufs=2, space="PSUM"))
                        _proj_qk(nc, tc, prj, t2_t, sa_wq, cb["sa_bq"], qT, ps)
                        _proj_qk(nc, tc, prj, t2_t, sa_wk, cb["sa_bk"], kT, ps)
                        _proj_v(nc, tc, prj, t2_t, sa_wv, bv_rep, v_aug, psv)
                with ExitStack() as phc:
                    _phi_k_side(nc, tc, phc, kT, v_aug, cb["sa_wf"],
                                c_negblk, ident_bf, kvT_t, s_t)
                with ExitStack() as phc:
                    _phi_q_side(nc, tc, phc, qT, cb["sa_wf"], c_negblk,
                                ident_bf, kvT_t, attn_t, c_eps16)
                with ExitStack() as woc:
                    _wo_ar(nc, tc, woc, attn_t, sa_wo, cb["sa_bo"],
                           ccs["in1"], ccs["out1"])
                sa_s.close()

                # ---------- CA kv side from memory (overlaps AR1)
                ca_s = ExitStack()
                wca_p = ca_s.enter_context(tc.tile_pool(name="wca", bufs=1))
                ca_wq = load_w(wca_p, wg["ca_wq"], "ca_wq")
                ca_wk = load_w(wca_p, wg["ca_wk"], "ca_wk")
                ca_wv = load_w(wca_p, wg["ca_wv"], "ca_wv")
                ca_wo = load_w(wca_p, wg["ca_wo"], "ca_wo")
                ca_kv = ca_s.enter_context(tc.tile_pool(name="cakv", bufs=1))
                kT2 = ca_kv.tile([128, 4, S], bf16, name="ca_kT")
                v_aug2 = ca_kv.tile([128, RT, HL, 65], bf16, name="ca_vaug")
                nc.vector.memset(v_aug2[:, :, :, 64:65], 1.0)
                kvT2 = ca_kv.tile([128, 4, 2, 2, 65], bf16, name="ca_kvT")
                s_t2 = ca_kv.tile([128, 4, 2], f32, name="ca_s_t")
                bv_rep2 = ca_kv.tile([128, 512], f32, name="ca_bvrep")
                nc.gpsimd.partition_broadcast(bv_rep2[:], cb["ca_bv"][:],
                                              channels=128)
                with ExitStack() as ph:
                    memp = ph.enter_context(tc.tile_pool(name="memp", bufs=1))
                    mem_t = memp.tile([128, KD, S], bf16, name="mem_t")
                    for kd in range(KD):
                        nc.sync.dma_start(
                            out=mem_t[:, kd, :],
                            in_=memT[kd * 128:(kd + 1) * 128, :])
                    with ExitStack() as prj:
                        ps = prj.enter_context(
                            tc.tile_pool(name="cprjps", bufs=1, space="PSUM"))
                        psv = prj.enter_context(
                            tc.tile_pool(name="cprjpsv", bufs=2, space="PSUM"))
                        _proj_qk(nc, tc, prj, mem_t, ca_wk, cb["ca_bk"],
                                 kT2, ps)
                        _proj_v(nc, tc, prj, mem_t, ca_wv, bv_rep2,
                                v_aug2, psv)
                with ExitStack() as phc:
                    _phi_k_side(nc, tc, phc, kT2, v_aug2, cb["ca_wf"],
                                c_negblk, ident_bf, kvT2, s_t2)

                # ---------- join AR1 -> LN2 -> CA q side
                caq_s = ExitStack()
                ca_q = caq_s.enter_context(tc.tile_pool(name="caq", bufs=1))
                qT2 = ca_q.tile([128, 4, S], bf16, name="ca_qT")
                attn2 = ca_q.tile([128, 4, S], bf16, name="ca_attn")
                for half in range(2):
                    with ExitStack() as ph:
                        t2p = ph.enter_context(tc.tile_pool(name="t2p2",
                                                            bufs=1))
                        t2h = t2p.tile([128, KD, SH], bf16, name="t2h")
                        with ExitStack() as js:
                            _join(nc, tc, js, x_t, ccs["out1"], half)
                        with ExitStack() as lns:
                            _ln2h(nc, tc, lns, x_t, cb["lng2"], cb["lnb2"],
                                  t2h, c_invd, c_eps, half)
                        with ExitStack() as prj:
                            ps = prj.enter_context(
                                tc.tile_pool(name="qprjps", bufs=1,
                                             space="PSUM"))
                            _proj_qk(nc, tc, prj, t2h, ca_wq, cb["ca_bq"],
                                     qT2, ps, s_off=half * SH, s_len=SH)
                with ExitStack() as phc:
                    _phi_q_side(nc, tc, phc, qT2, cb["ca_wf"], c_negblk,
                                ident_bf, kvT2, attn2, c_eps16)
                with ExitStack() as woc:
                    _wo_ar(nc, tc, woc, attn2, ca_wo, cb["ca_bo"],
                           ccs["in2"], ccs["out2"])
                caq_s.close()
                ca_s.close()

                # ---------- join AR2 -> LN3 -> FFN
                ffn_s = ExitStack()
                w12_p = ffn_s.enter_context(tc.tile_pool(name="w12", bufs=1))
                w1_sb = load_w(w12_p, w1_d, "w1_sb")
                w2_sb = load_w(w12_p, w2_d, "w2_sb")
                t2b = ffn_s.enter_context(
                    tc.tile_pool(name="t2p3", bufs=1)).tile(
                        [128, KD, S], bf16, name="t2b")
                for half in range(2):
                    with ExitStack() as js:
                        _join(nc, tc, js, x_t, ccs["out2"], half)
                    with ExitStack() as lns:
                        _ln(nc, tc, lns, x_t, cb["lng3"], cb["lnb3"], t2b,
                            c_invd, c_eps,
                            chunks=range(2 * half, 2 * half + 2))
                with ExitStack() as ph:
                    ps1 = ph.enter_context(tc.tile_pool(name="f1ps", bufs=2,
                                                        space="PSUM"))
                    ps2 = ph.enter_context(tc.tile_pool(name="f2ps", bufs=2,
                                                        space="PSUM"))
                    sb = ph.enter_context(tc.tile_pool(name="ffsb", bufs=2))
                    h1 = ph.enter_context(tc.tile_pool(name="h1p", bufs=1)) \
                        .tile([128, 16, 1024], bf16, name="h1")
                    for rh in range(2):
                        for mf in range(16):
                            h_ps = ps1.tile([128, 2, 512], f32, tag="f1",
                                            name="f1_ps")
                            for kd in range(KD):
                                for c2 in range(2):
                                    cs = bass.ds(rh * SH + c2 * 512, 512)
                                    nc.tensor.matmul(
                                        h_ps[:, c2, :],
                                        w1_sb[:, kd,
                                              mf * 128:(mf + 1) * 128],
                                        t2b[:, kd, cs],
                                        start=(kd == 0),
                                        stop=(kd == KD - 1),
                                        skip_group_check=True)
                            for c2 in range(2):
                                nc.scalar.activation(
                                    h1[:, mf, c2 * 512:(c2 + 1) * 512],
                                    h_ps[:, c2, :], AF.Relu,
                                    bias=cb["b1"][:, mf:mf + 1])
                        for md in range(KD):
                            o_ps = ps2.tile([128, 2, 512], f32, tag="f2",
                                            name="f2_ps")
                            for kf in range(16):
                                for c2 in range(2):
                                    nc.tensor.matmul(
                                        o_ps[:, c2, :],
                                        w2_sb[:, kf, md * 128:(md + 1) * 128],
                                        h1[:, kf, c2 * 512:(c2 + 1) * 512],
                                        start=(kf == 0), stop=(kf == 15),
                                        skip_group_check=True)
                            ev = sb.tile([128, 1024], bf16, tag="f2e",
                                         name="f2_ev")
                            for c2 in range(2):
                                nc.scalar.activation(
                                    ev[:, c2 * 512:(c2 + 1) * 512],
                                    o_ps[:, c2, :], AF.Identity,
                                    bias=cb["b2"][:, md:md + 1])
                            nc.sync.dma_start(
                                out=ccs["in3"][md * 128:(md + 1) * 128,
                                               bass.ds(rh * SH, SH)],
                                in_=ev[:])
                    _allreduce(nc, ccs["in3"], ccs["out3"])
                ffn_s.close()

                # ---------- out = x2 + AR3
                with ExitStack() as ph:
                    sb = ph.enter_context(tc.tile_pool(name="p10", bufs=3))
                    for half in range(2):
                        hs = bass.ds(half * SH, SH)
                        for kd in range(KD):
                            ks = slice(kd * 128, (kd + 1) * 128)
                            ar = sb.tile([128, SH], bf16, tag="ar", name="ar10")
                            nc.sync.dma_start(out=ar[:],
                                              in_=ccs["out3"][ks, hs])
                            xo = sb.tile([128, SH], f32, tag="xo", name="xo10")
                            nc.vector.tensor_add(xo[:], x_t[:, kd, hs], ar[:])
                            nc.sync.dma_start(out=outT[ks, hs], in_=xo[:])
                xs_stack.close()
    nc.finalize()
    return nc


# ------------------------------------------------------------------ host

def _prep_inputs(inputs):
    Cs = DH ** -0.25
    f = np.float32
    bf = ml_dtypes.bfloat16
    inp = {k: np.asarray(v, dtype=f) for k, v in inputs.items()}

    def fshape(vec):
        n = vec.shape[0] // 128
        return np.ascontiguousarray(vec.reshape(n, 128).T)

    def wstack(wT, k):
        # (128*k, n) -> (128, k, n) partition-major
        n = wT.shape[1]
        return np.ascontiguousarray(
            wT.reshape(k, 128, n).transpose(1, 0, 2)).astype(bf)

    consts = {}
    consts["c_invd"] = np.full((128, 128), 1.0 / D, bf)
    blk = np.zeros((128, 2), f)
    blk[0:64, 0] = -C2
    blk[64:128, 1] = -C2
    consts["c_negblk"] = blk.astype(bf)
    consts["c_ident"] = np.eye(128, dtype=bf)

    # fold LN affine (g, b) into the consuming projections:
    #   y = t2n@ (W*g).T + (b_w + W@b_ln)  with t2n = (x-mu)*rstd
    g1, b1n = inp["ln1_g"], inp["ln1_b"]
    g2, b2n = inp["ln2_g"], inp["ln2_b"]
    g3, b3n = inp["ln3_g"], inp["ln3_b"]
    eff = {}
    for nm, g_, bl in (("sa_wq", g1, b1n), ("sa_wk", g1, b1n),
                       ("sa_wv", g1, b1n), ("ca_wq", g2, b2n)):
        eff[nm] = inp[nm] * g_
        eff[nm + "_b"] = inp[nm.replace("w", "b")] + inp[nm] @ bl
    eff["ff_w1"] = inp["ff_w1"] * g3
    eff["ff_b1"] = inp["ff_b1"] + inp["ff_w1"] @ b3n

    in_maps = []
    for core in range(8):
        b, half = core // 2, core % 2
        hs = slice(half * 512, (half + 1) * 512)
        fs = slice(half * FL, (half + 1) * FL)
        m = dict(consts)
        m["xT"] = np.ascontiguousarray(inp["tgt"][:, b, :].T).astype(bf)
        m["memT"] = np.ascontiguousarray(inp["memory"][:, b, :].T).astype(bf)
        for pre in ("sa", "ca"):
            wq_f = eff[pre + "_wq"]
            bq_f = eff[pre + "_wq_b"]
            if pre == "sa":
                wk_f, bk_f = eff["sa_wk"], eff["sa_wk_b"]
                wv_f, bv_f = eff["sa_wv"], eff["sa_wv_b"]
            else:
                wk_f, bk_f = inp["ca_wk"], inp["ca_bk"]
                wv_f, bv_f = inp["ca_wv"], inp["ca_bv"]
            m[pre + "_wq"] = wstack(wq_f.T[:, hs], KD)
            m[pre + "_wk"] = wstack(wk_f.T[:, hs], KD)
            m[pre + "_wv"] = wstack(wv_f.T[:, hs], KD)
            m[pre + "_wo"] = wstack(np.ascontiguousarray(
                inp[pre + "_wo"].T[hs, :]), 4)
            m[pre + "_bq"] = fshape(bq_f[hs])
            m[pre + "_bk"] = fshape(bk_f[hs])
            m[pre + "_bv"] = bv_f[hs].reshape(1, 512).copy()
            m[pre + "_bo"] = fshape(inp[pre + "_bo"] * 0.5)
            m[pre + "_wf"] = np.ascontiguousarray(
                (Cs * inp[pre + "_feat"]).T).astype(bf)
        m["w1"] = wstack(np.ascontiguousarray(eff["ff_w1"].T[:, fs]), KD)
        m["w2"] = wstack(np.ascontiguousarray(inp["ff_w2"].T[fs, :]), 16)
        m["b1"] = fshape(eff["ff_b1"][fs])
        m["b2"] = fshape(inp["ff_b2"] * 0.5)
        for i in ("1", "2", "3"):
            m["ln%s_g" % i] = fshape(inp["ln%s_g" % i])
            m["ln%s_b" % i] = fshape(inp["ln%s_b" % i])
        in_maps.append(m)
    return in_maps


def _build_exec(nc, n_cores=8):
    import jax
    import jax.numpy as jnp
    from jax.sharding import Mesh, PartitionSpec
    from jax.experimental.shard_map import shard_map
    from concourse import bass2jax as b2j

    b2j.install_neuronx_cc_hook()
    partition_name = (nc.partition_id_tensor.name
                      if nc.partition_id_tensor else None)
    in_names, out_names, out_avals = [], [], []
    for alloc in nc.m.functions[0].allocations:
        if not isinstance(alloc, mybir.MemoryLocationSet):
            continue
        name = alloc.memorylocations[0].name
        if alloc.kind == "ExternalInput":
            if name != partition_name:
                in_names.append(name)
        elif alloc.kind == "ExternalOutput":
            out_names.append(name)
            out_avals.append(jax.core.ShapedArray(
                tuple(alloc.tensor_shape), mybir.dt.np(alloc.dtype)))
    n_params = len(in_names)
    all_in = list(in_names) + list(out_names)
    if partition_name is not None:
        all_in.append(partition_name)

    def _body(*args):
        operands = list(args)
        if partition_name is not None:
            operands.append(b2j.partition_id_tensor())
        outs = b2j._bass_exec_p.bind(
            *operands, out_avals=tuple(out_avals), in_names=tuple(all_in),
            out_names=tuple(out_names), lowering_input_output_aliases=(),
            sim_require_finite=True, sim_require_nnan=True, nc=nc)
        return tuple(outs)

    devices = jax.devices()[:n_cores]
    mesh = Mesh(np.asarray(devices), ("core",))
    n_outs = len(out_names)
    specs = (PartitionSpec("core"),) * (n_params + n_outs)
    out_specs = (PartitionSpec("core"),) * n_outs
    donate = tuple(range(n_params, n_params + n_outs))
    sharded = jax.jit(shard_map(_body, mesh=mesh, in_specs=specs,
                                out_specs=out_specs, check_rep=False),
                      donate_argnums=donate, keep_unused=True)

    def run(in_maps, fetch=True):
        import jax as _jax
        concat = [np.concatenate([np.asarray(in_maps[c][nm])
                                  for c in range(n_cores)], axis=0)
                  for nm in in_names]
        zeros = [np.zeros((n_cores * av.shape[0], *av.shape[1:]), av.dtype)
                 for av in out_avals]
        outs = sharded(*concat, *zeros)
        if not fetch:
            _jax.block_until_ready(outs)
            return None
        return [{nm: np.asarray(outs[i]).reshape(
            n_cores, *out_avals[i].shape)[c]
            for i, nm in enumerate(out_names)} for c in range(n_cores)]

    def time_exec(in_maps, iters=8):
        """Wall-time the sharded exec with device-resident inputs."""
        import time as _time
        import jax as _jax
        from jax.sharding import NamedSharding
        sh = NamedSharding(mesh, PartitionSpec("core"))
        concat = [np.concatenate([np.asarray(in_maps[c][nm])
                                  for c in range(n_cores)], axis=0)
                  for nm in in_names]
        dev_in = _jax.device_put(concat, [sh] * len(concat))
        _jax.block_until_ready(dev_in)
        zeros = [np.zeros((n_cores * av.shape[0], *av.shape[1:]), av.dtype)
                 for av in out_avals]
        times = []
        for _ in range(iters):
            zd = _jax.device_put(zeros, [sh] * len(zeros))
            _jax.block_until_ready(zd)
            t0 = _time.time()
            outs = sharded(*dev_in, *zd)
            _jax.block_until_ready(outs)
            times.append(_time.time() - t0)
        return times

    run.in_names = in_names
    run.time_exec = time_exec
    return run


def _get_exec():
    if "exec" not in _CACHE:
        nc = build_nc()
        _CACHE["exec"] = _build_exec(nc)
    return _CACHE["exec"]


def kernel(**inputs):
    run = _get_exec()
    in_maps = _prep_inputs(inputs)
    res = run(in_maps)
    out = np.empty((S, B, D), np.float32)
    for b in range(B):
        out[:, b, :] = res[2 * b]["outT"].T
    return out


# revision 28
# speedup vs baseline: 1.4582x; 1.4582x over previous
"""Trainium2 Bass kernel for nn_CustomDecoderLayer (FAVOR+ decoder layer).

Sharding: 8 cores = 4 batch groups x 2-way tensor parallel (heads/ffn),
pair all-reduces after Wo and W2. Per-core program is SPMD-identical.
Activations are F-layout on chip (features on partitions, seq on free dim).

v2: all weights bf16 + SBUF-resident (one DMA each), weight-stationary
matmul loops (Ldweights dedup), bf16 residual trunk kept in SBUF, 2-way
chunked all-reduces with CA memory-side (k/v + phi_k + kv summary)
overlapped with AR1.
"""
import os
import sys
sys.path.insert(0, "/opt/trn_rl_repo")
from contextlib import ExitStack

import numpy as np
import ml_dtypes

import concourse.bass as bass
import concourse.mybir as mybir
import concourse.tile as tile
from concourse import bacc, bass_isa

f32 = mybir.dt.float32
f32r = mybir.dt.float32r
bf16 = mybir.dt.bfloat16
AF = mybir.ActivationFunctionType
AX = mybir.AxisListType
ALU = mybir.AluOpType

D, H, DH, M = 1024, 16, 64, 256
S, B, F = 2048, 4, 4096
HL, FL = 8, 2048
C2 = 0.5 * (DH ** -0.5)      # 0.0625, exact in bf16
EPS16 = 1.0e-6 * 16.0
KD = D // 128                # 8
NCH = 4                      # col chunks of 512
RT = S // 128                # 16
SH = S // 2                  # 1024, AR chunk width
RG = [[0, 1], [2, 3], [4, 5], [6, 7]]

_CACHE = {}
KREPS = int(os.environ.get("KREPS", "1"))
TLSIM = os.environ.get("TLSIM", "0") == "1"


def _allreduce(nc, cc_in, cc_out):
    """Pair AllReduce; under TLSIM replaced by a DRAM copy (timing sim only)."""
    if TLSIM:
        nc.sync.dma_start(out=cc_out[:], in_=cc_in[:])
    else:
        nc.gpsimd.collective_compute("AllReduce", ALU.add, replica_groups=RG,
                                     ins=[cc_in.opt()], outs=[cc_out.opt()])


def _ln(nc, tc, ctx, x_t, g_t, b_t, out_t, c_invd, c_eps, chunks=range(NCH),
        out_off=0):
    """LayerNorm F-layout: x_t (128, KD, S) bf16 -> out_t bf16.

    Reads x_t chunks (global 512-col indices); writes out_t at column
    ch*512 - out_off (so a half-sized out tile starts at 0)."""
    ps = ctx.enter_context(tc.tile_pool(name="lnps", bufs=2, space="PSUM"))
    sb = ctx.enter_context(tc.tile_pool(name="lnsb", bufs=2))
    for ch in chunks:
        cs = bass.ts(ch, 512)
        os_ = bass.ds(ch * 512 - out_off, 512)
        mv = ps.tile([128, 2, 512], f32, tag="ln_ps", name="ln_ps")
        for kd in range(KD):
            nc.tensor.matmul(mv[:, 0, :], c_invd[:], x_t[:, kd, cs],
                             start=(kd == 0), stop=(kd == KD - 1),
                             skip_group_check=True)
        for kd in range(KD):
            x2 = sb.tile([128, 512], bf16, tag="ln_x2", name="ln_x2")
            nc.gpsimd.tensor_mul(x2[:], x_t[:, kd, cs], x_t[:, kd, cs])
            nc.tensor.matmul(mv[:, 1, :], c_invd[:], x2[:],
                             start=(kd == 0), stop=(kd == KD - 1),
                             skip_group_check=True)
        mu = sb.tile([128, 512], bf16, tag="ln_mu", name="ln_mu")
        nc.scalar.copy(mu[:], mv[:, 0, :])
        mu2 = sb.tile([128, 512], f32, tag="ln_mu2", name="ln_mu2")
        nc.vector.tensor_mul(mu2[:], mu[:], mu[:])
        var = sb.tile([128, 512], f32, tag="ln_var", name="ln_var")
        nc.vector.tensor_sub(var[:], mv[:, 1, :], mu2[:])
        lv = sb.tile([128, 512], f32, tag="ln_lv", name="ln_lv")
        nc.scalar.activation(lv[:], var[:], AF.Ln, bias=c_eps[:])
        rstd = sb.tile([128, 512], bf16, tag="ln_rstd", name="ln_rstd")
        nc.scalar.activation(rstd[:], lv[:], AF.Exp, scale=-0.5)
        for kd in range(KD):
            xm = sb.tile([128, 512], bf16, tag="ln_xm", name="ln_xm")
            nc.vector.tensor_sub(xm[:], x_t[:, kd, cs], mu[:])
            nc.vector.tensor_mul(out_t[:, kd, os_], xm[:], rstd[:])


def _ln2h(nc, tc, ctx, x_t, g_t, b_t, out_h, c_invd, c_eps, half):
    """LN of S-half `half` of x_t into half-width out tile (128, KD, SH)."""
    _ln(nc, tc, ctx, x_t, g_t, b_t, out_h, c_invd, c_eps,
        chunks=range(2 * half, 2 * half + 2), out_off=half * SH)


def _proj_qk(nc, tc, ctx, src_t, w_sb, b_t, out_t, ps, s_off=0, s_len=S):
    """out_t[:, mt, s_off:s_off+s_len] bf16 = (src.T @ W).T over s range.

    src_t is (128, KD, s_len); w_sb (128, KD, 512) bf16 resident."""
    nch = s_len // 512
    for mt in range(4):
        o_ps = ps.tile([128, nch, 512], f32, tag="proj_ps", name="proj_ps")
        for kd in range(KD):
            for ch in range(nch):
                nc.tensor.matmul(o_ps[:, ch, :],
                                 w_sb[:, kd, mt * 128:(mt + 1) * 128],
                                 src_t[:, kd, bass.ts(ch, 512)],
                                 start=(kd == 0), stop=(kd == KD - 1),
                                 skip_group_check=True)
        for ch in range(nch):
            nc.scalar.activation(
                out_t[:, mt, bass.ds(s_off + ch * 512, 512)],
                o_ps[:, ch, :], AF.Identity, bias=b_t[:, mt:mt + 1])


def _proj_v(nc, tc, ctx, src_t, wv_sb, bv_rep, v_aug, ps):
    """v R-layout -> v_aug (128, RT, HL, 65) bf16 (last col preset to 1)."""
    for rt in range(RT):
        v_ps = ps.tile([128, 512], f32, tag="v_ps", name="v_ps")
        for kd in range(KD):
            nc.tensor.matmul(v_ps[:], src_t[:, kd, rt * 128:(rt + 1) * 128],
                             wv_sb[:, kd, :], start=(kd == 0),
                             stop=(kd == KD - 1), skip_group_check=True)
        nc.vector.tensor_add(v_aug[:, rt, :, 0:64], v_ps[:], bv_rep[:])


def _phi_k_side(nc, tc, ctx, kT, v_aug, wf_pair, c_negblk, ident_bf, kvT_t,
                s_t):
    """FAVOR+ k side: E = exp(proj - sq) unstabilized in f32 PSUM; the global
    stab scale e^{-max} and the eps*sum(v) term are applied at the tiny kva
    stage, matching the reference eps weighting exactly."""
    ps_pj = ctx.enter_context(tc.tile_pool(name="kps_pj", bufs=3, space="PSUM"))
    ps_ns = ctx.enter_context(tc.tile_pool(name="kps_ns", bufs=2, space="PSUM"))
    ps_kv = ctx.enter_context(tc.tile_pool(name="kps_kv", bufs=2, space="PSUM"))
    ps_tp = ctx.enter_context(tc.tile_pool(name="kps_tp", bufs=1, space="PSUM"))
    sb = ctx.enter_context(tc.tile_pool(name="kphisb", bufs=6))
    big = ctx.enter_context(tc.tile_pool(name="kphibig", bufs=1))

    for p in range(4):
        E_t = big.tile([128, RT, 2, 257], bf16, tag="E_t", name="E_t")
        nc.vector.memset(E_t[:, :, :, 256:257], EPS16)
        stab_run = sb.tile([128, 2], f32, tag="stab_run", name="stab_run")
        for rt in range(RT):
            rs = bass.ts(rt, 128)
            k2 = sb.tile([128, 128], bf16, tag="k2", name="k2")
            nc.gpsimd.tensor_mul(k2[:], kT[:, p, rs], kT[:, p, rs])
            pj = ps_pj.tile([128, 2, 256], f32, tag="pj", name="pj_k")
            nc.tensor.matmul(pj[:, :, :], kT[:, p, rs], wf_pair[:])
            nsq = ps_ns.tile([128, 2], f32, tag="nsq", name="nsq_k")
            nc.tensor.matmul(nsq[:], k2[:], c_negblk[:])
            rmax = sb.tile([128, 2], f32, tag="rmax", name="rmax_k")
            nc.vector.reduce_max(rmax[:], pj[:], axis=AX.X)
            if rt == 0:
                nc.vector.tensor_copy(stab_run[:], rmax[:])
            else:
                nc.vector.tensor_max(stab_run[:], stab_run[:], rmax[:])
            nsq_sb = sb.tile([128, 2], f32, tag="nsq_sb", name="nsq_sb")
            nc.scalar.copy(nsq_sb[:], nsq[:])
            for h in range(2):
                nc.scalar.activation(E_t[:, rt, h, 0:256], pj[:, h, :],
                                     AF.Exp, bias=nsq_sb[:, h:h + 1])
        stab_rep = sb.tile([128, 2], f32, tag="stab_rep", name="stab_rep")
        nc.gpsimd.partition_all_reduce(stab_rep[:], stab_run[:], channels=128,
                                       reduce_op=bass_isa.ReduceOp.max)
        nc.scalar.activation(s_t[:, p, :], stab_rep[:], AF.Exp, scale=-1.0)
        for h in range(2):
            kv_ps = ps_kv.tile([65, 257], f32, tag="kv_ps", name="kv_ps")
            for rt in range(RT):
                nc.tensor.matmul(kv_ps[:], v_aug[:, rt, p * 2 + h, :],
                                 E_t[:, rt, h, :], start=(rt == 0),
                                 stop=(rt == RT - 1), skip_group_check=True)
            csum = sb.tile([65, 1], f32, tag="csum", name="csum")
            nc.vector.tensor_copy(csum[:], kv_ps[:, 256:257])
            kva = sb.tile([65, 256], bf16, tag="kva", name="kva")
            nc.vector.tensor_scalar(kva[:], kv_ps[:, 0:256],
                                    s_t[0:65, p, h:h + 1], csum[:],
                                    ALU.mult, ALU.add)
            for mt in range(2):
                tp = ps_tp.tile([128, 65], bf16, tag="tp_kv", name="tp_kv")
                nc.tensor.transpose(tp[:], kva[0:65, mt * 128:(mt + 1) * 128],
                                    ident_bf[0:65, 0:65])
                nc.scalar.copy(kvT_t[:, p, h, mt, :], tp[:])


def _phi_q_side(nc, tc, ctx, qT, wf_pair, c_negblk, ident_bf, kvT_t, attn_t,
                c_eps16):
    """FAVOR+ q side: phi(q), out/z -> attn_t (128, 4, S) bf16."""
    ps_pj = ctx.enter_context(tc.tile_pool(name="qps_pj", bufs=2, space="PSUM"))
    ps_ns = ctx.enter_context(tc.tile_pool(name="qps_ns", bufs=1, space="PSUM"))
    ps_tp = ctx.enter_context(tc.tile_pool(name="qps_tp", bufs=1, space="PSUM"))
    ps_o = ctx.enter_context(tc.tile_pool(name="qps_o", bufs=3, space="PSUM"))
    sb = ctx.enter_context(tc.tile_pool(name="qphisb", bufs=6))
    big = ctx.enter_context(tc.tile_pool(name="qphibig", bufs=2))

    for p in range(4):
        pqT = big.tile([128, 2, 2, S], bf16, tag="pqT", name="pqT")
        for rt in range(RT):
            rs = bass.ts(rt, 128)
            q2 = sb.tile([128, 128], bf16, tag="q2", name="q2")
            nc.gpsimd.tensor_mul(q2[:], qT[:, p, rs], qT[:, p, rs])
            pj = ps_pj.tile([128, 2, 256], f32, tag="pj", name="pj_q")
            nc.tensor.matmul(pj[:, :, :], qT[:, p, rs], wf_pair[:])
            nsq = ps_ns.tile([128, 2], f32, tag="nsq", name="nsq_q")
            nc.tensor.matmul(nsq[:], q2[:], c_negblk[:])
            rmax = sb.tile([128, 2], f32, tag="rmax", name="rmax_q")
            nc.vector.reduce_max(rmax[:], pj[:], axis=AX.X)
            bias_q = sb.tile([128, 2], f32, tag="bias_q", name="bias_q")
            nc.vector.tensor_sub(bias_q[:], nsq[:], rmax[:])
            pqR = sb.tile([128, 2, 256], bf16, tag="pqR", name="pqR")
            for h in range(2):
                nc.scalar.activation(pqR[:, h, :], pj[:, h, :], AF.Exp,
                                     bias=bias_q[:, h:h + 1])
            for h in range(2):
                for mt in range(2):
                    tp = ps_tp.tile([128, 128], bf16, tag="tp_pq",
                                    name="tp_pq")
                    nc.tensor.transpose(tp[:],
                                        pqR[:, h, mt * 128:(mt + 1) * 128],
                                        ident_bf[:])
                    if mt == 0:
                        nc.vector.tensor_scalar_add(pqT[:, h, mt, rs], tp[:],
                                                    EPS16)
                    else:
                        nc.scalar.activation(pqT[:, h, mt, rs], tp[:],
                                             AF.Identity, bias=c_eps16[:])
        for ch in range(NCH):
            cs = bass.ts(ch, 512)
            for h in range(2):
                hp = slice(64 * h, 64 * h + 64)
                o_ps = ps_o.tile([65, 512], f32, tag="o_ps", name="o_ps")
                for mt in range(2):
                    nc.tensor.matmul(o_ps[:, :], kvT_t[:, p, h, mt, 0:65],
                                     pqT[:, h, mt, cs], start=(mt == 0),
                                     stop=(mt == 1), skip_group_check=True)
                zr = sb.tile([1, 512], f32, tag="zr", name="zr")
                nc.vector.reciprocal(zr[:], o_ps[64:65, :])
                zb = sb.tile([128, 512], f32, tag="zb", name="zb")
                nc.gpsimd.partition_broadcast(zb[:], zr[:], channels=128)
                nc.vector.tensor_mul(attn_t[hp, p, cs], o_ps[0:64, :],
                                     zb[0:64, :])


def _wo_ar(nc, tc, ctx, attn_t, wo_sb, bo_t, cc_i, cc_o):
    """Wo projection, per S-half: write cc_in half, AllReduce it."""
    ps = ctx.enter_context(tc.tile_pool(name="wops", bufs=2, space="PSUM"))
    sb = ctx.enter_context(tc.tile_pool(name="wosb", bufs=2))
    for half in range(2):
        for md in range(KD):
            o_ps = ps.tile([128, 2, 512], f32, tag="wo_ps", name="wo_ps")
            for k4 in range(4):
                for c2 in range(2):
                    cs = bass.ds(half * SH + c2 * 512, 512)
                    nc.tensor.matmul(o_ps[:, c2, :],
                                     wo_sb[:, k4, md * 128:(md + 1) * 128],
                                     attn_t[:, k4, cs],
                                     start=(k4 == 0), stop=(k4 == 3),
                                     skip_group_check=True)
            ev = sb.tile([128, 1024], bf16, tag="wo_ev", name="wo_ev")
            for c2 in range(2):
                nc.scalar.activation(ev[:, c2 * 512:(c2 + 1) * 512],
                                     o_ps[:, c2, :], AF.Identity,
                                     bias=bo_t[:, md:md + 1])
            nc.sync.dma_start(
                out=cc_i[md * 128:(md + 1) * 128,
                         bass.ds(half * SH, SH)],
                in_=ev[:])
    _allreduce(nc, cc_i, cc_o)


def _join(nc, tc, ctx, x_t, cc_o, half):
    """x_t[:, :, half] += AR output half (bf16)."""
    sb = ctx.enter_context(tc.tile_pool(name="joinsb", bufs=3))
    hs = bass.ds(half * SH, SH)
    for kd in range(KD):
        ar = sb.tile([128, SH], bf16, tag="jar", name="jar")
        nc.sync.dma_start(out=ar[:], in_=cc_o[kd * 128:(kd + 1) * 128, hs])
        nc.vector.tensor_add(x_t[:, kd, hs], x_t[:, kd, hs], ar[:])


def build_nc():
    nc = bacc.Bacc("TRN2", target_bir_lowering=False, debug=False,
                   num_devices=8)

    def din(name, shape, dtype=f32):
        return nc.dram_tensor(name, list(shape), dtype,
                              kind="ExternalInput").ap()

    xT = din("xT", (D, S), bf16)
    memT = din("memT", (D, S), bf16)
    wg = {}
    for pre in ("sa", "ca"):
        wg[pre + "_wq"] = din(pre + "_wq", (128, KD, 512), bf16)
        wg[pre + "_wk"] = din(pre + "_wk", (128, KD, 512), bf16)
        wg[pre + "_wv"] = din(pre + "_wv", (128, KD, 512), bf16)
        wg[pre + "_wo"] = din(pre + "_wo", (128, 4, D), bf16)
        wg[pre + "_bq"] = din(pre + "_bq", (128, 4))
        wg[pre + "_bk"] = din(pre + "_bk", (128, 4))
        wg[pre + "_bv"] = din(pre + "_bv", (1, 512))
        wg[pre + "_bo"] = din(pre + "_bo", (128, 8))
        wg[pre + "_wf"] = din(pre + "_wf", (64, 256), bf16)
    w1_d = din("w1", (128, KD, FL), bf16)
    w2_d = din("w2", (128, 16, D), bf16)
    b1_d = din("b1", (128, 16))
    b2_d = din("b2", (128, 8))
    ln_d = {}
    for i in ("1", "2", "3"):
        ln_d["g" + i] = din("ln%s_g" % i, (128, 8))
        ln_d["b" + i] = din("ln%s_b" % i, (128, 8))
    c_invd_d = din("c_invd", (128, 128), bf16)
    c_negblk_d = din("c_negblk", (128, 2), bf16)
    ident_d = din("c_ident", (128, 128), bf16)

    outT = nc.dram_tensor("outT", [D, S], bf16, kind="ExternalOutput").ap()

    with tile.TileContext(nc) as tc:
        with ExitStack() as top:
            dram = top.enter_context(tc.tile_pool(name="dram", bufs=1,
                                                  space="DRAM"))
            ccs = {}
            for i in ("1", "2", "3"):
                ccs["in" + i] = dram.tile([D, S], bf16, name="cc_in" + i)
                ccs["out" + i] = dram.tile([D, S], bf16, name="cc_out" + i)

            const = top.enter_context(tc.tile_pool(name="const", bufs=1))
            c_invd = const.tile([128, 128], bf16, name="c_invd")
            nc.sync.dma_start(out=c_invd[:], in_=c_invd_d[:])
            c_negblk = const.tile([128, 2], bf16, name="c_negblk")
            nc.sync.dma_start(out=c_negblk[:], in_=c_negblk_d[:])
            ident_bf = const.tile([128, 128], bf16, name="ident_bf")
            nc.sync.dma_start(out=ident_bf[:], in_=ident_d[:])
            c_eps = const.tile([128, 1], f32, name="c_eps")
            nc.vector.memset(c_eps[:], 1.0e-5)
            c_eps16 = const.tile([128, 1], f32, name="c_eps16")
            nc.vector.memset(c_eps16[:], EPS16)
            cb = {}
            for pre in ("sa", "ca"):
                for nm in ("bq", "bk", "bo"):
                    t = const.tile(list(wg[pre + "_" + nm].shape), f32,
                                   name=pre + nm)
                    nc.sync.dma_start(out=t[:], in_=wg[pre + "_" + nm][:])
                    cb[pre + "_" + nm] = t
                t = const.tile([1, 512], f32, name=pre + "bv")
                nc.sync.dma_start(out=t[:], in_=wg[pre + "_bv"][:])
                cb[pre + "_bv"] = t
                wff = const.tile([128, 512], bf16, name=pre + "wff")
                nc.vector.memset(wff[64:128, 0:256], 0.0)
                nc.vector.memset(wff[0:64, 256:512], 0.0)
                nc.sync.dma_start(out=wff[0:64, 0:256], in_=wg[pre + "_wf"][:])
                nc.sync.dma_start(out=wff[64:128, 256:512],
                                  in_=wg[pre + "_wf"][:])
                cb[pre + "_wf"] = wff
            for nm, d_ap in (("b1", b1_d), ("b2", b2_d)):
                t = const.tile(list(d_ap.shape), f32, name=nm)
                nc.sync.dma_start(out=t[:], in_=d_ap[:])
                cb[nm] = t
            for k, d_ap in ln_d.items():
                t = const.tile([128, 8], f32, name="ln" + k)
                nc.sync.dma_start(out=t[:], in_=d_ap[:])
                cb["ln" + k] = t

            def load_w(pool, d_ap, name):
                t = pool.tile(list(d_ap.shape), bf16, name=name)
                nc.sync.dma_start(out=t[:], in_=d_ap[:])
                return t

            for _rep in range(KREPS):
                # ---------- residual trunk, resident bf16
                xs_stack = ExitStack()
                x_t = xs_stack.enter_context(
                    tc.tile_pool(name="xpool", bufs=1, side="right")).tile(
                        [128, KD, S], bf16, name="x_t")
                for kd in range(KD):
                    nc.sync.dma_start(out=x_t[:, kd, :],
                                      in_=xT[kd * 128:(kd + 1) * 128, :])

                # ---------- LN1 -> t2 ; SA block
                sa_s = ExitStack()
                wsa_p = sa_s.enter_context(tc.tile_pool(name="wsa", bufs=1))
                sa_wq = load_w(wsa_p, wg["sa_wq"], "sa_wq")
                sa_wk = load_w(wsa_p, wg["sa_wk"], "sa_wk")
                sa_wv = load_w(wsa_p, wg["sa_wv"], "sa_wv")
                sa_wo = load_w(wsa_p, wg["sa_wo"], "sa_wo")
                sa_res = sa_s.enter_context(tc.tile_pool(name="sares", bufs=1))
                qT = sa_res.tile([128, 4, S], bf16, name="sa_qT")
                kT = sa_res.tile([128, 4, S], bf16, name="sa_kT")
                v_aug = sa_res.tile([128, RT, HL, 65], bf16, name="sa_vaug")
                nc.vector.memset(v_aug[:, :, :, 64:65], 1.0)
                attn_t = sa_res.tile([128, 4, S], bf16, name="sa_attn")
                kvT_t = sa_res.tile([128, 4, 2, 2, 65], bf16, name="sa_kvT")
                s_t = sa_res.tile([128, 4, 2], f32, name="sa_s_t")
                bv_rep = sa_res.tile([128, 512], f32, name="sa_bvrep")
                nc.gpsimd.partition_broadcast(bv_rep[:], cb["sa_bv"][:],
                                              channels=128)
                with ExitStack() as ph:
                    t2p = ph.enter_context(tc.tile_pool(name="t2p1", bufs=1,
                                                        side="right"))
                    t2_t = t2p.tile([128, KD, S], bf16, name="t2_t1")
                    with ExitStack() as lns:
                        _ln(nc, tc, lns, x_t, cb["lng1"], cb["lnb1"], t2_t,
                            c_invd, c_eps)
                    with ExitStack() as prj:
                        ps = prj.enter_context(
                            tc.tile_pool(name="prjps", bufs=1, space="PSUM"))
                        psv = prj.enter_context(
                            tc.tile_pool(name="prjpsv", bufs=2, space="PSUM"))
                        _proj_qk(nc, tc, prj, t2_t, sa_wq, cb["sa_bq"], qT, ps)
                        _proj_qk(nc, tc, prj, t2_t, sa_wk, cb["sa_bk"], kT, ps)
                        _proj_v(nc, tc, prj, t2_t, sa_wv, bv_rep, v_aug, psv)
                with ExitStack() as phc:
                    _phi_k_side(nc, tc, phc, kT, v_aug, cb["sa_wf"],
                                c_negblk, ident_bf, kvT_t, s_t)
                with ExitStack() as phc:
                    _phi_q_side(nc, tc, phc, qT, cb["sa_wf"], c_negblk,
                                ident_bf, kvT_t, attn_t, c_eps16)
                with ExitStack() as woc:
                    _wo_ar(nc, tc, woc, attn_t, sa_wo, cb["sa_bo"],
                           ccs["in1"], ccs["out1"])
                sa_s.close()

                # ---------- CA kv side from memory (overlaps AR1)
                ca_s = ExitStack()
                wca_p = ca_s.enter_context(tc.tile_pool(name="wca", bufs=1))
                ca_wq = load_w(wca_p, wg["ca_wq"], "ca_wq")
                ca_wk = load_w(wca_p, wg["ca_wk"], "ca_wk")
                ca_wv = load_w(wca_p, wg["ca_wv"], "ca_wv")
                ca_wo = load_w(wca_p, wg["ca_wo"], "ca_wo")
                ca_kv = ca_s.enter_context(tc.tile_pool(name="cakv", bufs=1))
                kT2 = ca_kv.tile([128, 4, S], bf16, name="ca_kT")
                v_aug2 = ca_kv.tile([128, RT, HL, 65], bf16, name="ca_vaug")
                nc.vector.memset(v_aug2[:, :, :, 64:65], 1.0)
                kvT2 = ca_kv.tile([128, 4, 2, 2, 65], bf16, name="ca_kvT")
                s_t2 = ca_kv.tile([128, 4, 2], f32, name="ca_s_t")
                bv_rep2 = ca_kv.tile([128, 512], f32, name="ca_bvrep")
                nc.gpsimd.partition_broadcast(bv_rep2[:], cb["ca_bv"][:],
                                              channels=128)
                with ExitStack() as ph:
                    memp = ph.enter_context(tc.tile_pool(name="memp", bufs=1))
                    mem_t = memp.tile([128, KD, S], bf16, name="mem_t")
                    for kd in range(KD):
                        nc.sync.dma_start(
                            out=mem_t[:, kd, :],
                            in_=memT[kd * 128:(kd + 1) * 128, :])
                    with ExitStack() as prj:
                        ps = prj.enter_context(
                            tc.tile_pool(name="cprjps", bufs=1, space="PSUM"))
                        psv = prj.enter_context(
                            tc.tile_pool(name="cprjpsv", bufs=2, space="PSUM"))
                        _proj_qk(nc, tc, prj, mem_t, ca_wk, cb["ca_bk"],
                                 kT2, ps)
                        _proj_v(nc, tc, prj, mem_t, ca_wv, bv_rep2,
                                v_aug2, psv)
                with ExitStack() as phc:
                    _phi_k_side(nc, tc, phc, kT2, v_aug2, cb["ca_wf"],
                                c_negblk, ident_bf, kvT2, s_t2)

                # ---------- join AR1 -> LN2 -> CA q side
                caq_s = ExitStack()
                ca_q = caq_s.enter_context(tc.tile_pool(name="caq", bufs=1))
                qT2 = ca_q.tile([128, 4, S], bf16, name="ca_qT")
                attn2 = ca_q.tile([128, 4, S], bf16, name="ca_attn")
                for half in range(2):
                    with ExitStack() as ph:
                        t2p = ph.enter_context(tc.tile_pool(name="t2p2",
                                                            bufs=1))
                        t2h = t2p.tile([128, KD, SH], bf16, name="t2h")
                        with ExitStack() as js:
                            _join(nc, tc, js, x_t, ccs["out1"], half)
                        with ExitStack() as lns:
                            _ln2h(nc, tc, lns, x_t, cb["lng2"], cb["lnb2"],
                                  t2h, c_invd, c_eps, half)
                        with ExitStack() as prj:
                            ps = prj.enter_context(
                                tc.tile_pool(name="qprjps", bufs=1,
                                             space="PSUM"))
                            _proj_qk(nc, tc, prj, t2h, ca_wq, cb["ca_bq"],
                                     qT2, ps, s_off=half * SH, s_len=SH)
                with ExitStack() as phc:
                    _phi_q_side(nc, tc, phc, qT2, cb["ca_wf"], c_negblk,
                                ident_bf, kvT2, attn2, c_eps16)
                with ExitStack() as woc:
                    _wo_ar(nc, tc, woc, attn2, ca_wo, cb["ca_bo"],
                           ccs["in2"], ccs["out2"])
                caq_s.close()
                ca_s.close()

                # ---------- join AR2 -> LN3 -> FFN
                ffn_s = ExitStack()
                w12_p = ffn_s.enter_context(tc.tile_pool(name="w12", bufs=1))
                w1_sb = load_w(w12_p, w1_d, "w1_sb")
                w2_sb = load_w(w12_p, w2_d, "w2_sb")
                t2b = ffn_s.enter_context(
                    tc.tile_pool(name="t2p3", bufs=1)).tile(
                        [128, KD, S], bf16, name="t2b")
                for half in range(2):
                    with ExitStack() as js:
                        _join(nc, tc, js, x_t, ccs["out2"], half)
                    with ExitStack() as lns:
                        _ln(nc, tc, lns, x_t, cb["lng3"], cb["lnb3"], t2b,
                            c_invd, c_eps,
                            chunks=range(2 * half, 2 * half + 2))
                with ExitStack() as ph:
                    ps1 = ph.enter_context(tc.tile_pool(name="f1ps", bufs=2,
                                                        space="PSUM"))
                    ps2 = ph.enter_context(tc.tile_pool(name="f2ps", bufs=2,
                                                        space="PSUM"))
                    sb = ph.enter_context(tc.tile_pool(name="ffsb", bufs=2))
                    h1 = ph.enter_context(tc.tile_pool(name="h1p", bufs=1)) \
                        .tile([128, 16, 1024], bf16, name="h1")
                    for rh in range(2):
                        for mf in range(16):
                            h_ps = ps1.tile([128, 2, 512], f32, tag="f1",
                                            name="f1_ps")
                            for kd in range(KD):
                                for c2 in range(2):
                                    cs = bass.ds(rh * SH + c2 * 512, 512)
                                    nc.tensor.matmul(
                                        h_ps[:, c2, :],
                                        w1_sb[:, kd,
                                              mf * 128:(mf + 1) * 128],
                                        t2b[:, kd, cs],
                                        start=(kd == 0),
                                        stop=(kd == KD - 1),
                                        skip_group_check=True)
                            for c2 in range(2):
                                nc.scalar.activation(
                                    h1[:, mf, c2 * 512:(c2 + 1) * 512],
                                    h_ps[:, c2, :], AF.Relu,
                                    bias=cb["b1"][:, mf:mf + 1])
                        for md in range(KD):
                            o_ps = ps2.tile([128, 2, 512], f32, tag="f2",
                                            name="f2_ps")
                            for kf in range(16):
                                for c2 in range(2):
                                    nc.tensor.matmul(
                                        o_ps[:, c2, :],
                                        w2_sb[:, kf, md * 128:(md + 1) * 128],
                                        h1[:, kf, c2 * 512:(c2 + 1) * 512],
                                        start=(kf == 0), stop=(kf == 15),
                                        skip_group_check=True)
                            ev = sb.tile([128, 1024], bf16, tag="f2e",
                                         name="f2_ev")
                            for c2 in range(2):
                                nc.scalar.activation(
                                    ev[:, c2 * 512:(c2 + 1) * 512],
                                    o_ps[:, c2, :], AF.Identity,
                                    bias=cb["b2"][:, md:md + 1])
                            nc.sync.dma_start(
                                out=ccs["in3"][md * 128:(md + 1) * 128,
                                               bass.ds(rh * SH, SH)],
                                in_=ev[:])
                    _allreduce(nc, ccs["in3"], ccs["out3"])
                ffn_s.close()

                # ---------- out = x2 + AR3
                with ExitStack() as ph:
                    sb = ph.enter_context(tc.tile_pool(name="p10", bufs=3))
                    for half in range(2):
                        hs = bass.ds(half * SH, SH)
                        for kd in range(KD):
                            ks = slice(kd * 128, (kd + 1) * 128)
                            ar = sb.tile([128, SH], bf16, tag="ar", name="ar10")
                            nc.sync.dma_start(out=ar[:],
                                              in_=ccs["out3"][ks, hs])
                            xo = sb.tile([128, SH], bf16, tag="xo", name="xo10")
                            nc.vector.tensor_add(xo[:], x_t[:, kd, hs], ar[:])
                            nc.sync.dma_start(out=outT[ks, hs], in_=xo[:])
                xs_stack.close()
    nc.finalize()
    return nc


# ------------------------------------------------------------------ host

def _prep_inputs(inputs):
    Cs = DH ** -0.25
    f = np.float32
    bf = ml_dtypes.bfloat16
    inp = {k: np.asarray(v, dtype=f) for k, v in inputs.items()}

    def fshape(vec):
        n = vec.shape[0] // 128
        return np.ascontiguousarray(vec.reshape(n, 128).T)

    def wstack(wT, k):
        # (128*k, n) -> (128, k, n) partition-major
        n = wT.shape[1]
        return np.ascontiguousarray(
            wT.reshape(k, 128, n).transpose(1, 0, 2)).astype(bf)

    consts = {}
    consts["c_invd"] = np.full((128, 128), 1.0 / D, bf)
    blk = np.zeros((128, 2), f)
    blk[0:64, 0] = -C2
    blk[64:128, 1] = -C2
    consts["c_negblk"] = blk.astype(bf)
    consts["c_ident"] = np.eye(128, dtype=bf)

    # fold LN affine (g, b) into the consuming projections:
    #   y = t2n@ (W*g).T + (b_w + W@b_ln)  with t2n = (x-mu)*rstd
    g1, b1n = inp["ln1_g"], inp["ln1_b"]
    g2, b2n = inp["ln2_g"], inp["ln2_b"]
    g3, b3n = inp["ln3_g"], inp["ln3_b"]
    eff = {}
    for nm, g_, bl in (("sa_wq", g1, b1n), ("sa_wk", g1, b1n),
                       ("sa_wv", g1, b1n), ("ca_wq", g2, b2n)):
        eff[nm] = inp[nm] * g_
        eff[nm + "_b"] = inp[nm.replace("w", "b")] + inp[nm] @ bl
    eff["ff_w1"] = inp["ff_w1"] * g3
    eff["ff_b1"] = inp["ff_b1"] + inp["ff_w1"] @ b3n

    in_maps = []
    for core in range(8):
        b, half = core // 2, core % 2
        hs = slice(half * 512, (half + 1) * 512)
        fs = slice(half * FL, (half + 1) * FL)
        m = dict(consts)
        m["xT"] = np.ascontiguousarray(inp["tgt"][:, b, :].T).astype(bf)
        m["memT"] = np.ascontiguousarray(inp["memory"][:, b, :].T).astype(bf)
        for pre in ("sa", "ca"):
            wq_f = eff[pre + "_wq"]
            bq_f = eff[pre + "_wq_b"]
            if pre == "sa":
                wk_f, bk_f = eff["sa_wk"], eff["sa_wk_b"]
                wv_f, bv_f = eff["sa_wv"], eff["sa_wv_b"]
            else:
                wk_f, bk_f = inp["ca_wk"], inp["ca_bk"]
                wv_f, bv_f = inp["ca_wv"], inp["ca_bv"]
            m[pre + "_wq"] = wstack(wq_f.T[:, hs], KD)
            m[pre + "_wk"] = wstack(wk_f.T[:, hs], KD)
            m[pre + "_wv"] = wstack(wv_f.T[:, hs], KD)
            m[pre + "_wo"] = wstack(np.ascontiguousarray(
                inp[pre + "_wo"].T[hs, :]), 4)
            m[pre + "_bq"] = fshape(bq_f[hs])
            m[pre + "_bk"] = fshape(bk_f[hs])
            m[pre + "_bv"] = bv_f[hs].reshape(1, 512).copy()
            m[pre + "_bo"] = fshape(inp[pre + "_bo"] * 0.5)
            m[pre + "_wf"] = np.ascontiguousarray(
                (Cs * inp[pre + "_feat"]).T).astype(bf)
        m["w1"] = wstack(np.ascontiguousarray(eff["ff_w1"].T[:, fs]), KD)
        m["w2"] = wstack(np.ascontiguousarray(inp["ff_w2"].T[fs, :]), 16)
        m["b1"] = fshape(eff["ff_b1"][fs])
        m["b2"] = fshape(inp["ff_b2"] * 0.5)
        for i in ("1", "2", "3"):
            m["ln%s_g" % i] = fshape(inp["ln%s_g" % i])
            m["ln%s_b" % i] = fshape(inp["ln%s_b" % i])
        in_maps.append(m)
    return in_maps


def _build_exec(nc, n_cores=8):
    import jax
    import jax.numpy as jnp
    from jax.sharding import Mesh, PartitionSpec
    from jax.experimental.shard_map import shard_map
    from concourse import bass2jax as b2j

    b2j.install_neuronx_cc_hook()
    partition_name = (nc.partition_id_tensor.name
                      if nc.partition_id_tensor else None)
    in_names, out_names, out_avals = [], [], []
    for alloc in nc.m.functions[0].allocations:
        if not isinstance(alloc, mybir.MemoryLocationSet):
            continue
        name = alloc.memorylocations[0].name
        if alloc.kind == "ExternalInput":
            if name != partition_name:
                in_names.append(name)
        elif alloc.kind == "ExternalOutput":
            out_names.append(name)
            out_avals.append(jax.core.ShapedArray(
                tuple(alloc.tensor_shape), mybir.dt.np(alloc.dtype)))
    n_params = len(in_names)
    all_in = list(in_names) + list(out_names)
    if partition_name is not None:
        all_in.append(partition_name)

    def _body(*args):
        operands = list(args)
        if partition_name is not None:
            operands.append(b2j.partition_id_tensor())
        outs = b2j._bass_exec_p.bind(
            *operands, out_avals=tuple(out_avals), in_names=tuple(all_in),
            out_names=tuple(out_names), lowering_input_output_aliases=(),
            sim_require_finite=True, sim_require_nnan=True, nc=nc)
        return tuple(outs)

    devices = jax.devices()[:n_cores]
    mesh = Mesh(np.asarray(devices), ("core",))
    n_outs = len(out_names)
    specs = (PartitionSpec("core"),) * (n_params + n_outs)
    out_specs = (PartitionSpec("core"),) * n_outs
    donate = tuple(range(n_params, n_params + n_outs))
    sharded = jax.jit(shard_map(_body, mesh=mesh, in_specs=specs,
                                out_specs=out_specs, check_rep=False),
                      donate_argnums=donate, keep_unused=True)

    def run(in_maps, fetch=True):
        import jax as _jax
        concat = [np.concatenate([np.asarray(in_maps[c][nm])
                                  for c in range(n_cores)], axis=0)
                  for nm in in_names]
        zeros = [np.zeros((n_cores * av.shape[0], *av.shape[1:]), av.dtype)
                 for av in out_avals]
        outs = sharded(*concat, *zeros)
        if not fetch:
            _jax.block_until_ready(outs)
            return None
        return [{nm: np.asarray(outs[i]).reshape(
            n_cores, *out_avals[i].shape)[c]
            for i, nm in enumerate(out_names)} for c in range(n_cores)]

    def time_exec(in_maps, iters=8):
        """Wall-time the sharded exec with device-resident inputs."""
        import time as _time
        import jax as _jax
        from jax.sharding import NamedSharding
        sh = NamedSharding(mesh, PartitionSpec("core"))
        concat = [np.concatenate([np.asarray(in_maps[c][nm])
                                  for c in range(n_cores)], axis=0)
                  for nm in in_names]
        dev_in = _jax.device_put(concat, [sh] * len(concat))
        _jax.block_until_ready(dev_in)
        zeros = [np.zeros((n_cores * av.shape[0], *av.shape[1:]), av.dtype)
                 for av in out_avals]
        times = []
        for _ in range(iters):
            zd = _jax.device_put(zeros, [sh] * len(zeros))
            _jax.block_until_ready(zd)
            t0 = _time.time()
            outs = sharded(*dev_in, *zd)
            _jax.block_until_ready(outs)
            times.append(_time.time() - t0)
        return times

    run.in_names = in_names
    run.time_exec = time_exec
    return run


def _get_exec():
    if "exec" not in _CACHE:
        nc = build_nc()
        _CACHE["exec"] = _build_exec(nc)
    return _CACHE["exec"]


def kernel(**inputs):
    run = _get_exec()
    in_maps = _prep_inputs(inputs)
    res = run(in_maps)
    out = np.empty((S, B, D), np.float32)
    for b in range(B):
        out[:, b, :] = res[2 * b]["outT"].T.astype(np.float32)
    return out
